# revision 43
# baseline (speedup 1.0000x reference)
"""Trainium2 Bass kernel for single-head attention (nn_Attention_31344671326347).

Problem: B=4, S=2048, E=D=1024, fp32.
    q = x @ Wq.T + bq ; k = x @ Wk.T + bk ; v = x @ Wv.T + bv
    out = softmax(q k^T / sqrt(D)) @ v

Sharding: 8 cores = (4 batches) x (2 sequence-halves). Core (b, h) projects
Q, K and V for ITS OWN sequence half only (so each projection is computed
exactly once fleet-wide), then pair-wise DRAM AllGathers exchange the K and V
halves inside each batch pair. Attention (scores + PV) runs on the core's own
1024 query rows against the full gathered K/V. The gathers are split into two
pieces each and kicked as the projections produce them, so they ride under
the remaining projection compute.

All matmul operands are bf16 (same PE rate as fp32r, half the SBUF/DMA/
collective bytes); PSUM accumulation and the output path stay fp32. The
softmax denominator rides the PV matmul via 8 ones-columns appended to v
(wvT zero-padded + bv ones-padded), so no separate reduction pass is needed.

Layout: all contractions keep the contracted dim on SBUF partitions. Host
ships x^T (own half) and W^T so qT [d,s], kT [d,t] and v [t,d] come straight
out of the PE with zero on-device transposes; softmax runs over the partition
dim via exp (ScalarE) + the ones-column denominator.

Scheduling notes (what made this fast — PE idles ~3us of a ~220us run):
- Projection order K, V, Q: the K gathers (needed first, by scores) kick
  earliest, and V's eviction tail + collective traffic hide under Q.
- A 1-byte prelude AllGather (via the kernel-barrier machinery) performs
  the CC engine's ~40us one-time setup at kernel start; the four real
  gathers then run back-to-back on the serial CC engine well before use.
- CC triggers block the gpsimd queue until the previous collective
  completes, so bounce-buffer stores ride the Scalar queue instead, and
  kst/vs staging lives in the outermost pool scope so their loads fire on
  gather completion rather than on an inner pool-scope barrier.
- V-projection PSUM eviction runs on Scalar (Vector couldn't keep pace);
  bv for chunks 0/1 is applied in the PV epilogue using sum(p)=1, and
  chunk 2 (which carries the ones-columns) keeps a Vector bias-add whose
  bv slice is normalization-invariant.
- A throwaway matmul group on memset scratch runs during the ~11us boot
  window (no DMA dependency) to pre-ramp the PE clock before the first
  projection; the last PV group is split in two so half its epilogue
  hides under matmuls.
"""

import numpy as np
import ml_dtypes

import concourse.bass as bass
import concourse.mybir as mybir
import concourse.tile as tile
from concourse import bacc
from concourse.bass_utils import run_bass_kernel_spmd

B, S, E, D = 4, 2048, 1024, 1024
SQ = S // 2          # query rows per core == key/value rows projected per core
P = 128
EO = E // P          # 8 contraction chunks
DO = D // P          # 8 d chunks
TC = S // P          # 16 key/t chunks (full sequence)
DA = 1032            # d + 8 ones columns (denominator rides the PV matmul)
DC = 344             # PV d-chunk width (3 * 344 = 1032)
F32 = mybir.dt.float32
BF16 = mybir.dt.bfloat16
GROUPS = [[0, 1], [2, 3], [4, 5], [6, 7]]

N_CORES = 8
TRACE = False        # test.py flips this for profiling
LAST_RESULT = None   # BassKernelResults of the most recent run

_NC = None


def _build():
    nc = bacc.Bacc("TRN2", target_bir_lowering=False, debug=False,
                   num_devices=N_CORES)

    xT = nc.dram_tensor("xT", [E, SQ], BF16, kind="ExternalInput")
    wqT = nc.dram_tensor("wqT", [E, D], BF16, kind="ExternalInput")
    wkT = nc.dram_tensor("wkT", [E, D], BF16, kind="ExternalInput")
    wvT = nc.dram_tensor("wvT", [E, DA], BF16, kind="ExternalInput")
    bq = nc.dram_tensor("bq", [P, DO], F32, kind="ExternalInput")
    bk = nc.dram_tensor("bk", [P, DO], F32, kind="ExternalInput")
    bv = nc.dram_tensor("bv", [P, DA], F32, kind="ExternalInput")
    out = nc.dram_tensor("out", [SQ, D], F32, kind="ExternalOutput")

    xT_r = xT.rearrange("(eo p) s -> p eo s", p=P)
    wq_r = wqT.rearrange("(eo p) d -> p eo d", p=P)
    wk_r = wkT.rearrange("(eo p) d -> p eo d", p=P)
    wv_r = wvT.rearrange("(eo p) d -> p eo d", p=P)

    Ident = mybir.ActivationFunctionType.Identity
    Exp = mybir.ActivationFunctionType.Exp
    inv_sqrt_d = float(1.0 / np.sqrt(D))

    with tile.TileContext(nc) as tc:
        with (
            tc.tile_pool(name="res", bufs=1) as res,
            tc.tile_pool(name="small", bufs=1) as small,
            tc.tile_pool(name="kst", bufs=2) as kstp,
            tc.tile_pool(name="psP", bufs=8, space="PSUM") as psP,
            tc.tile_pool(name="dram", bufs=1, space="DRAM") as dpool,
        ):
            qT_t = res.tile([P, DO, SQ], BF16, tag="qT")
            # eT[t, s] for both query halves: chunk index = sb*TC + t_chunk
            eT_t = res.tile([P, 2 * TC, 512], BF16, tag="eT")

            bqk = small.tile([P, 2 * DO], F32, tag="bqk")
            bq_t = bqk[:, :DO]
            bk_t = bqk[:, DO:]
            bv_t = small.tile([P, DA], F32, tag="bv")

            # Collective bounce buffers. k piece tb covers my-half t columns
            # [tb*512, (tb+1)*512); v piece pv covers my-half rows likewise.
            # Gather outputs are rank-major: block r = core r's piece, i.e.
            # global t block r*1024 + piece*512 — identical on both cores of
            # a pair, so the program stays rank-symmetric (SPMD-safe).
            k_in = [dpool.tile([D, 512], BF16, tag=f"kin{i}", name=f"kin{i}")
                    for i in range(2)]
            k_out = [dpool.tile([2, D, 512], BF16, tag=f"kout{i}",
                                name=f"kout{i}") for i in range(2)]
            v_in = [dpool.tile([512, DA], BF16, tag=f"vin{i}", name=f"vin{i}")
                    for i in range(2)]
            v_out = [dpool.tile([2, 512, DA], BF16, tag=f"vout{i}",
                                name=f"vout{i}") for i in range(2)]
            # kst/vs staging live in the outer scope: their loads must
            # fire the moment the gathers complete, not when an inner pool
            # scope's SBUF becomes free (pool-open barriers would serialize
            # the vs loads behind the whole scores phase).
            vs_t = res.tile([P, TC, DA], BF16, tag="vs", name="vs_t")



            # ---- projections (each core: own sequence half only) ----
            with (
                tc.tile_pool(name="wpool", bufs=3) as wpool,
                tc.tile_pool(name="xs", bufs=1) as xsp,
                tc.tile_pool(name="kb", bufs=8) as kbp,
                tc.tile_pool(name="vb", bufs=6) as vbp,
            ):
                w_q = wpool.tile([P, EO, D], BF16, tag="w", name="w_q")
                w_k = wpool.tile([P, EO, D], BF16, tag="w", name="w_k")
                w_v = wpool.tile([P, EO, DA], BF16, tag="w", name="w_v")
                xs0 = xsp.tile([P, EO, 512], BF16, tag="x0", name="xs0")
                xs1 = xsp.tile([P, EO, 512], BF16, tag="x1", name="xs1")

                # PE clock warm-up: the tensor engine idles ~11us during
                # boot while the first DMAs land, then pays a pstate ramp
                # tax (~+15% on the first projection). Run a throwaway
                # accumulation group on memset scratch — no DMA dependency,
                # so it executes during the boot window and pre-ramps the
                # clock before real work arrives.
                warm_t = small.tile([P, 512], BF16, tag="warm",
                                    name="warm_t")
                nc.vector.memset(warm_t[:], 0.0)
                wps = psP.tile([P, 512], F32, tag="ps", name="wps")
                for i in range(6):
                    nc.tensor.matmul(
                        wps[:],
                        warm_t[:, 0:P], warm_t[:],
                        start=(i == 0), stop=(i == 5),
                    )
                nc.scalar.activation(warm_t[:], wps[:], Ident)

                # startup-critical DMAs first: the minimal working set of
                # the very first matmul (xs0 eo0 + the first 128 w_k
                # columns), then incrementally larger pieces so the
                # eo-outer matmul rows track DMA arrival.
                nc.sync.dma_start(xs0[:, 0:1, :], xT_r[:, 0:1, 0:512])
                nc.gpsimd.dma_start(w_k[:, 0:1, :], wk_r[:, 0:1, :])
                nc.sync.dma_start(xs0[:, 1:2, :], xT_r[:, 1:2, 0:512])
                nc.gpsimd.dma_start(w_k[:, 1:2, :], wk_r[:, 1:2, :])
                for eo in range(2, EO, 2):
                    nc.sync.dma_start(
                        xs0[:, eo:eo + 2, :], xT_r[:, eo:eo + 2, 0:512])
                nc.gpsimd.dma_start(w_k[:, 2:EO, :], wk_r[:, 2:EO, :])
                nc.sync.dma_start(xs1[:], xT_r[:, :, 512:1024])
                nc.gpsimd.dma_start(bqk[:, :DO], bq[:])
                nc.gpsimd.dma_start(bqk[:, DO:], bk[:])
                nc.gpsimd.dma_start(bv_t[:], bv[:])
                nc.sync.dma_start(w_v[:], wv_r[:])
                nc.sync.dma_start(w_q[:], wq_r[:])

                # K projection first so the K AllGathers kick as early as
                # possible. kT[d, t] for my half; each 512-col piece goes to
                # DRAM and is AllGathered across the pair immediately.
                # do-outer throughout: each group consumes the w_k chunks
                # progressively (so the first matmul still only needs eo=0),
                # and the evictions stagger one per group instead of
                # bunching 8 at once on the Scalar engine.
                for tb in range(2):
                    xk = xs0 if tb == 0 else xs1
                    for do in range(DO):
                        ps = psP.tile([P, 512], F32, tag="ps")
                        for eo in range(EO):
                            nc.tensor.matmul(
                                ps[:], w_k[:, eo, do * P:(do + 1) * P],
                                xk[:, eo, :],
                                start=(eo == 0), stop=(eo == EO - 1),
                            )
                        kb = kbp.tile([P, 512], BF16, tag="kb")
                        nc.scalar.activation(
                            kb[:], ps[:], Ident, bias=bk_t[:, do:do + 1])
                        nc.scalar.dma_start(
                            k_in[tb][do * P:(do + 1) * P, :], kb[:])
                    nc.gpsimd.collective_compute(
                        "AllGather", mybir.AluOpType.bypass,
                        replica_groups=GROUPS,
                        ins=[k_in[tb].opt()], outs=[k_out[tb].opt()],
                    )

                # V projection: v[t, d] for my half. Since softmax weights
                # sum to 1, sum_t p_t (v_t + bv) = sum_t p_t v_t + bv, so
                # bv for chunks 0/1 moves to the PV epilogue and their
                # evictions run on the otherwise-idle Scalar engine —
                # keeping PSUM recycling off the critical path (the Vector
                # engine couldn't keep pace with the matmuls). Chunk 2
                # keeps its bias add: it carries the ones-columns (softmax
                # denominator), and its bv slice is normalization-invariant.
                for tci in range(8):
                    xv = (xs0 if tci < 4 else xs1)[
                        :, :, (tci % 4) * P:(tci % 4 + 1) * P]
                    for ck in range(3):
                        ps = psP.tile([P, DC], F32, tag="ps")
                        for eo in range(EO):
                            nc.tensor.matmul(
                                ps[:], xv[:, eo, :],
                                w_v[:, eo, ck * DC:(ck + 1) * DC],
                                start=(eo == 0), stop=(eo == EO - 1),
                            )
                        vb = vbp.tile([P, DC], BF16, tag="vb")
                        if ck == 2:
                            nc.vector.tensor_add(
                                vb[:], ps[:], bv_t[:, 2 * DC:3 * DC])
                        else:
                            nc.scalar.activation(vb[:], ps[:], Ident)
                        nc.scalar.dma_start(
                            v_in[tci // 4][(tci % 4) * P:(tci % 4 + 1) * P,
                                           ck * DC:(ck + 1) * DC],
                            vb[:])
                    if tci == 3 or tci == 7:
                        nc.gpsimd.collective_compute(
                            "AllGather", mybir.AluOpType.bypass,
                            replica_groups=GROUPS,
                            ins=[v_in[tci // 4].opt()],
                            outs=[v_out[tci // 4].opt()],
                        )

                # Q projection: qT[d, s] = Wq @ x^T (+ bq per-partition).
                for sb in range(2):
                    xq = xs0 if sb == 0 else xs1
                    for do in range(DO):
                        ps = psP.tile([P, 512], F32, tag="ps")
                        for eo in range(EO):
                            nc.tensor.matmul(
                                ps[:], w_q[:, eo, do * P:(do + 1) * P],
                                xq[:, eo, :],
                                start=(eo == 0), stop=(eo == EO - 1),
                            )
                        nc.scalar.activation(
                            qT_t[:, do, sb * 512:(sb + 1) * 512], ps[:],
                            Ident, bias=bq_t[:, do:do + 1],
                        )

            # ---- scores: eT = exp((q kT)/sqrt(d)), by gathered k piece ----
            with (
                tc.tile_pool(name="sc_unused", bufs=1) as _scu,
            ):
                for tb in range(2):
                    ko_r = k_out[tb].rearrange("r (do p) t -> p r do t", p=P)
                    for r in range(2):
                        kst = kstp.tile([P, DO, 512], BF16, tag="kst")
                        nc.sync.dma_start(kst[:], ko_r[:, r, :, :])
                        for tcl in range(4):
                            tc_g = r * 8 + tb * 4 + tcl
                            for sb in range(2):
                                ps = psP.tile([P, 512], F32, tag="ps")
                                for do in range(DO):
                                    nc.tensor.matmul(
                                        ps[:],
                                        kst[:, do, tcl * P:(tcl + 1) * P],
                                        qT_t[:, do, sb * 512:(sb + 1) * 512],
                                        start=(do == 0), stop=(do == DO - 1),
                                    )
                                nc.scalar.activation(
                                    eT_t[:, sb * TC + tc_g, :], ps[:],
                                    Exp, scale=inv_sqrt_d)

            # ---- PV: out = (eT^T @ v) / denominator ----
            # v (both halves) fully resident in SBUF; chunk ck=2 carries the
            # ones-columns, so its col D-2*DC is the softmax denominator.
            with (
                tc.tile_pool(name="ot", bufs=4) as otp,
                tc.tile_pool(name="rc", bufs=2) as rcp,
            ):
                # vs t-chunk layout: piece pv covers my-half rows, rank r
                # selects the pair half → natural t chunk r*8 + pv*4 + i.
                # PV accumulates piece-0's chunks first so piece-1's gather
                # gets extra slack.
                vs = vs_t
                for pv in range(2):
                    vo_r = v_out[pv].rearrange("r (t p) d -> p r t d", p=P)
                    for r in range(2):
                        blk = r * 8 + pv * 4
                        nc.sync.dma_start(
                            vs[:, blk:blk + 4, :], vo_r[:, r, :, :])
                tci_order = [0, 1, 2, 3, 8, 9, 10, 11,
                             4, 5, 6, 7, 12, 13, 14, 15]

                for sb in range(2):
                    for ss in range(4):
                        row0 = sb * 512 + ss * P
                        po2 = psP.tile([P, DC], F32, tag="ps")
                        for i, tci in enumerate(tci_order):
                            nc.tensor.matmul(
                                po2[:], eT_t[:, sb * TC + tci,
                                             ss * P:(ss + 1) * P],
                                vs[:, tci, 2 * DC:3 * DC],
                                start=(i == 0), stop=(i == TC - 1),
                            )
                        recip = rcp.tile([P, 1], F32, tag="rc")
                        nc.vector.reciprocal(
                            recip[:], po2[:, D - 2 * DC:D - 2 * DC + 1])
                        o2 = otp.tile([P, DC], F32, tag="ot")
                        w2 = D - 2 * DC
                        nc.vector.tensor_scalar_mul(
                            o2[:, :w2], po2[:, :w2], recip[:])
                        nc.gpsimd.dma_start(
                            out[row0:row0 + P, 2 * DC:D], o2[:, :w2])
                        for ck in range(2):
                            # the very last group runs as two 172-wide
                            # halves so half its epilogue hides under the
                            # second half's matmuls (shorter serial tail)
                            halves = ([(0, DC)] if not
                                      (sb == 1 and ss == 3 and ck == 1)
                                      else [(0, DC // 2), (DC // 2, DC)])
                            for (c0, c1) in halves:
                                w = c1 - c0
                                po = psP.tile([P, DC], F32, tag="ps")
                                for i, tci in enumerate(tci_order):
                                    nc.tensor.matmul(
                                        po[:, :w], eT_t[:, sb * TC + tci,
                                                        ss * P:(ss + 1) * P],
                                        vs[:, tci, ck * DC + c0:ck * DC + c1],
                                        start=(i == 0), stop=(i == TC - 1),
                                    )
                                o_t = otp.tile([P, DC], F32, tag="ot")
                                nc.vector.tensor_scalar_mul(
                                    o_t[:, :w], po[:, :w], recip[:])
                                nc.vector.tensor_add(
                                    o_t[:, :w], o_t[:, :w],
                                    bv_t[:, ck * DC + c0:ck * DC + c1])
                                nc.gpsimd.dma_start(
                                    out[row0:row0 + P,
                                        ck * DC + c0:ck * DC + c1],
                                    o_t[:, :w])

    # Request the 1-byte prelude AllGather (normally emitted for
    # bir_kernel_barrier_wait) WITHOUT any engine waiting on it: the CC
    # engine performs its one-time setup at kernel start instead of
    # delaying the first real gather by ~40us. A wait instruction inside
    # TileContext would deadlock the scheduler sim (the matching inc is
    # only inserted at compile), so only the prelude CC is requested.
    nc._bir_kernel_barrier_sem_replica_groups.extend(set(g) for g in GROUPS)
    nc.compile()
    return nc


def _get_nc():
    global _NC
    if _NC is None:
        _NC = _build()
    return _NC


def kernel(x, Wq, bq, Wk, bk, Wv, bv):
    global LAST_RESULT
    bf16 = ml_dtypes.bfloat16
    x = np.asarray(x, dtype=np.float32)
    Wq = np.asarray(Wq, dtype=np.float32)
    Wk = np.asarray(Wk, dtype=np.float32)
    Wv = np.asarray(Wv, dtype=np.float32)
    bq_ = np.asarray(bq, dtype=np.float32)
    bk_ = np.asarray(bk, dtype=np.float32)
    bv_ = np.asarray(bv, dtype=np.float32)

    wqT = np.ascontiguousarray(Wq.T).astype(bf16)
    wkT = np.ascontiguousarray(Wk.T).astype(bf16)
    wvT = np.zeros((E, DA), dtype=bf16)
    wvT[:, :D] = Wv.T.astype(bf16)
    bq_r = np.ascontiguousarray(bq_.reshape(DO, P).T)
    bk_r = np.ascontiguousarray(bk_.reshape(DO, P).T)
    bv_aug = np.concatenate([bv_, np.ones(DA - D, np.float32)])
    bv_r = np.ascontiguousarray(np.broadcast_to(bv_aug, (P, DA)))

    in_maps = []
    for c in range(N_CORES):
        b, h = divmod(c, 2)
        xTh = np.ascontiguousarray(
            x[b].T[:, h * SQ:(h + 1) * SQ]).astype(bf16)
        in_maps.append({
            "xT": xTh,
            "wqT": wqT, "wkT": wkT, "wvT": wvT,
            "bq": bq_r, "bk": bk_r, "bv": bv_r,
        })

    nc = _get_nc()
    res = run_bass_kernel_spmd(nc, in_maps, list(range(N_CORES)), trace=TRACE)
    LAST_RESULT = res

    out = np.empty((B, S, D), dtype=np.float32)
    for c in range(N_CORES):
        b, h = divmod(c, 2)
        out[b, h * SQ:(h + 1) * SQ, :] = res.results[c]["out"]
    return out
